# revision 18
# baseline (speedup 1.0000x reference)
"""Depthwise causal Conv1D (B=4, C=4096, L=4096, K=4) on 8 trn2 NeuronCores.

Sharding: channel-parallel (tensor parallel) — core i owns channels
[i*512, (i+1)*512). Depthwise conv has zero cross-channel interaction, so
there is no communication; each core computes its channel slab end to end.

All device I/O is fp16: the host casts x to fp16 before upload and
upconverts the fp16 output to fp32 after download (correctness budget is
rel_err < 2e-2; this path lands ~1e-3). That halves DMA traffic vs fp32
(~33.6 MB per core).

Per-core layout: channels on SBUF partitions (128 at a time), time on the
free dim; x tiles are zero-stuffed with PAD columns at both ends. The FIR
out[m] = sum_t w_t*xp[m+t] + bias is split by tap parity:

  PE      : taps 1,3 (odd shifts) via diagonal weight matmuls into PSUM,
            1024-col fp16 chunks (odd-element shifts would break the DVE
            16-bit packed fast path, but the PE streams 1 col/cycle
            regardless of alignment)
  ScalarE : out[chunk] = psum + 0   (merge; reads PSUM, writes fp16)
  VectorE : taps 0,2 (even shifts, 4B-aligned) as tensor_scalar (4x fp16)
            into tmp tiles + tensor_tensor adds (2x fp16); bias rides
            tap0's tensor_scalar second scalar. scalar_tensor_tensor is
            avoided entirely — it has no fast-path uop (1x always).

Queues: x loads on Sync HWDGE, out stores on GpSimd SWDGE, constants on
Scalar — store descriptor generation stays off the busy ACT sequencer.
First/last tiles are processed in 1024-col pieces to shorten ramp/drain.
"""

import numpy as np

import concourse.bass as bass
import concourse.tile as tile
from concourse import bacc, mybir
from concourse.bass_utils import run_bass_kernel_spmd

B, C, L, K = 4, 4096, 4096, 4
PAD = K - 1
LOUT = L + PAD  # 4099
NCORES = 8
CS = C // NCORES  # 512 channels per core
DT16 = mybir.dt.float16
DT32 = mybir.dt.float32

_AF = mybir.ActivationFunctionType
_OP = mybir.AluOpType

MM_CHUNK = 512  # psum/matmul chunk width (1 matmul pair per chunk, ISA max 512)


def build_nc(b=B, cs=CS, l=L, k=K, n_bufs=6):
    """Build the per-core Bass program (fp16 I/O, tap-parity engine split)."""
    ng = cs // 128
    pad = k - 1
    lout = l + pad
    wx = pad + l + pad  # 4102, even -> fp16 tiles stay 4B-aligned
    assert wx % 2 == 0

    nc = bacc.Bacc("TRN2", target_bir_lowering=False, debug=False, num_devices=NCORES)
    x_d = nc.dram_tensor("x", [b, cs, l], DT16, kind="ExternalInput").ap()
    # packed per-channel constants: wb[c] = [w_0..w_{k-1}, bias]
    wb_d = nc.dram_tensor("wb", [cs, k + 1], DT32, kind="ExternalInput").ap()
    eye_d = nc.dram_tensor("eye", [128, 128], DT16, kind="ExternalInput").ap()
    o_d = nc.dram_tensor("out", [b, cs, lout], DT16, kind="ExternalOutput").ap()

    with tile.TileContext(nc) as tc:
        with (
            tc.tile_pool(name="consts", bufs=1) as cpool,
            tc.tile_pool(name="xs", bufs=n_bufs + 1) as xpool,
            tc.tile_pool(name="os", bufs=n_bufs + 1) as opool,
            tc.tile_pool(name="tmps", bufs=5) as tpool,
            tc.tile_pool(name="ps", bufs=2, space="PSUM") as ppool,
        ):
            # Constants ride the Scalar queue (x loads own Sync).
            consts = []
            for g in range(ng):
                ct = cpool.tile([128, k + 1], DT32, tag=f"c{g}")
                nc.scalar.dma_start(ct[:], wb_d[g * 128 : (g + 1) * 128, :])
                consts.append(ct)
            ident = cpool.tile([128, 128], DT16, tag="eye")
            nc.scalar.dma_start(ident[:], eye_d[:])
            diags = {}
            for g in range(ng):
                for t in (1, 2, 3):
                    dg = cpool.tile([128, 128], DT16, tag=f"d{g}_{t}")
                    nc.vector.tensor_scalar(
                        out=dg[:], in0=ident[:],
                        scalar1=consts[g][:, t : t + 1],
                        scalar2=None, op0=_OP.mult,
                    )
                    diags[(g, t)] = dg

            n_tiles = b * ng

            GRP = 1536  # merge-group width: [128,1536] fp32 psum = 3 banks exactly

            def emit_pe_act(ot, xt, g, m_lo, m_hi, taps=(1, 2, 3)):
                """PE taps -> PSUM (512-col matmuls, ISA max) in merge groups;
                one ACT merge (psum -> fp16 ot) per group."""
                for g0 in range(m_lo, m_hi, GRP):
                    gn = min(GRP, m_hi - g0)
                    pt = ppool.tile([128, 1536], DT32, tag="p")
                    for s0 in range(0, gn, MM_CHUNK):
                        n = min(MM_CHUNK, gn - s0)
                        for i, t in enumerate(taps):
                            nc.tensor.matmul(
                                pt[:, s0 : s0 + n], lhsT=diags[(g, t)][:],
                                rhs=xt[:, g0 + s0 + t : g0 + s0 + t + n],
                                start=(i == 0), stop=(i == len(taps) - 1),
                            )
                    nc.scalar.activation(
                        ot[:, g0 : g0 + gn], pt[:, 0:gn], _AF.Identity,
                        bias=0.0, scale=1.0,
                    )

            def emit_dve(ot, xt, ct, ta, m_lo, m_hi, tb=None):
                """DVE tap 0 (+bias) — and tap 2 too when tb is given:
                tensor_scalar (4x fp16) + tensor_tensor (2x fp16) only."""
                nc.vector.tensor_scalar(
                    out=ta[:, m_lo:m_hi], in0=xt[:, m_lo:m_hi],
                    scalar1=ct[:, 0:1], scalar2=ct[:, k : k + 1],
                    op0=_OP.mult, op1=_OP.add,
                )
                if tb is not None:
                    nc.vector.tensor_scalar(
                        out=tb[:, m_lo:m_hi], in0=xt[:, m_lo + 2 : m_hi + 2],
                        scalar1=ct[:, 2:3], scalar2=None, op0=_OP.mult,
                    )
                    nc.vector.tensor_tensor(
                        out=ta[:, m_lo:m_hi], in0=ta[:, m_lo:m_hi],
                        in1=tb[:, m_lo:m_hi], op=_OP.add,
                    )
                nc.vector.tensor_tensor(
                    out=ot[:, m_lo:m_hi], in0=ot[:, m_lo:m_hi],
                    in1=ta[:, m_lo:m_hi], op=_OP.add,
                )

            # Interior tiles whose tap 2 runs on DVE instead of PE —
            # balances Tensor vs Vector engine occupancy.
            dve2_tiles = {3, 5, 8, 10, 12}

            ti = 0
            for bi in range(b):
                for g in range(ng):
                    c0 = g * 128
                    edge = ti == 0 or ti == n_tiles - 1
                    dve2 = ti in dve2_tiles
                    ct = consts[g]

                    xt = xpool.tile([128, wx], DT16, tag="x")
                    nc.gpsimd.memset(xt[:, 0:pad], 0.0)
                    nc.gpsimd.memset(xt[:, pad + l : wx], 0.0)
                    ot = opool.tile([128, lout + 1], DT16, tag="o")
                    ta = tpool.tile([128, lout + 1], DT16, tag="ta")
                    if dve2:
                        tb = tpool.tile([128, lout + 1], DT16, tag="tb")
                    else:
                        tb = None

                    if edge:
                        # Piece-wise load/compute/store: piece c covers out
                        # [w*c-pad, w*(c+1)-pad) so tap reads stay within x
                        # pieces <= c. Finer pieces on the first tile shorten
                        # the pipeline ramp; on the last tile they shorten the
                        # serial matmul->merge->add->store drain chain, and its
                        # stores use the Scalar HWDGE (~0.6us completion vs
                        # ~2us on the GpSimd SWDGE path).
                        last = ti == n_tiles - 1
                        npiece = 8 if last else 4
                        w = l // npiece
                        for c in range(npiece):
                            j0, j1 = c * w, (c + 1) * w
                            nc.sync.dma_start(
                                xt[:, pad + j0 : pad + j1],
                                x_d[bi, c0 : c0 + 128, j0:j1],
                            )
                            m_lo = 0 if c == 0 else j0 - pad
                            m_hi = lout if c == npiece - 1 else j1 - pad
                            emit_pe_act(ot, xt, g, m_lo, m_hi)
                            emit_dve(ot, xt, ct, ta, m_lo, m_hi)
                            st = nc.scalar if last else nc.gpsimd
                            st.dma_start(
                                o_d[bi, c0 : c0 + 128, m_lo:m_hi], ot[:, m_lo:m_hi]
                            )
                    else:
                        nc.sync.dma_start(
                            xt[:, pad : pad + l], x_d[bi, c0 : c0 + 128, :]
                        )
                        emit_pe_act(ot, xt, g, 0, lout, taps=(1, 3) if dve2 else (1, 2, 3))
                        emit_dve(ot, xt, ct, ta, 0, lout, tb=tb)
                        nc.gpsimd.dma_start(
                            o_d[bi, c0 : c0 + 128, :], ot[:, 0:lout]
                        )
                    ti += 1
    nc.compile()
    return nc


_cached_nc = None


def _get_nc():
    global _cached_nc
    if _cached_nc is None:
        _cached_nc = build_nc()
    return _cached_nc


def run(x, kernel, bias, trace=False, **kwargs):
    """Shard, run on 8 cores, gather. Returns (out, BassKernelResults)."""
    x16 = np.asarray(x).astype(np.float16)
    w = np.asarray(kernel, dtype=np.float32).reshape(K, C)
    bvec = np.asarray(bias, dtype=np.float32).reshape(C)
    # wb[c] = [w_0[c] .. w_{K-1}[c], bias[c]]
    wb = np.concatenate([w.T, bvec[:, None]], axis=1).astype(np.float32)

    eye = np.eye(128, dtype=np.float16)
    in_maps = []
    for i in range(NCORES):
        sl = slice(i * CS, (i + 1) * CS)
        in_maps.append(
            {
                "x": np.ascontiguousarray(x16[:, sl, :]),
                "wb": np.ascontiguousarray(wb[sl, :]),
                "eye": eye,
            }
        )

    nc = _get_nc()
    bkr = run_bass_kernel_spmd(
        nc, in_maps, core_ids=list(range(NCORES)), trace=trace, **kwargs
    )
    out = np.concatenate([r["out"] for r in bkr.results], axis=1).astype(np.float32)
    return out, bkr


def kernel(x, kernel, bias):
    import os

    prev = os.environ.get("BASS_NEVER_TRACE")
    os.environ["BASS_NEVER_TRACE"] = "1"  # keep the runner off the NTFF path
    try:
        out, _ = run(x, kernel, bias)
    finally:
        if prev is None:
            os.environ.pop("BASS_NEVER_TRACE", None)
        else:
            os.environ["BASS_NEVER_TRACE"] = prev
    return out
